# revision 2
# baseline (speedup 1.0000x reference)
"""Trainium2 Bass kernel for nn_Custom_Final_Pooling_2D (segment_reduce).

Computes out = einsum("rn,bn->br", T, x*x) where T is the fixed binary
2x2-pooling selector built by the reference's build_pooling_matrix(32, 16):
  - T has shape [496, 1024]; only rows r0(l)+c are nonzero, where
    r0(l) = 31*l - l*(l+1)//2 + 15, for l, c in [0, 16).
  - Row r0(l)+c sums x[.., i*32+j]^2 over the 2x2 window
    i in {2l, 2l+1}, j in {2c, 2c+1}.

So the kernel is: square (ScalarE, in place), pairwise add along j
(VectorE, stride-2), pairwise add along i (VectorE) into the dense
[rows, 256] pool result, then a contiguous DMA store of that dense
layout. The host scatters the 16 column blocks to offsets r0(l) and
materializes the 240 always-zero columns while gathering. (Writing the
496- or 361-wide layouts on device moves 27-48% more bytes; strided
partial-width stores measured ~1.9x slower per byte than contiguous.)

Data-parallel over 8 NeuronCores: batch dim sharded 65536 -> 8 x 8192.
"""

import numpy as np

import concourse.bacc as bacc
import concourse.mybir as mybir
from concourse.tile import TileContext
from concourse.bass_utils import run_bass_kernel_spmd

N_CORES = 8
BATCH = 65536
IMG = 32          # input image side
OUT_SIDE = 16     # pooled side
N_FEAT = IMG * IMG          # 1024
N_OUT = (2 * OUT_SIDE) * (2 * OUT_SIDE - 1) // 2  # 496
ROWS_PER_CORE = BATCH // N_CORES  # 8192

P = 128           # SBUF partitions
R = 8             # batch rows per partition per supertile
SUPER = P * R     # 1024 batch rows per supertile
N_TILES = ROWS_PER_CORE // SUPER  # 8

# Nonzero-row offsets of T: line l's 16 outputs live at columns
# r0(l) .. r0(l)+15 of the 496-wide output. Cols < 15 and >= 376 are
# always zero (as are the interior gaps); they stay at the memset value.
R0 = [31 * l - l * (l + 1) // 2 + 15 for l in range(OUT_SIDE)]

# The device writes the fully dense [rows, 256] pool output (line-major
# 16x16 blocks) — the exact nonzero values, contiguous, at full write
# bandwidth with 48% fewer bytes than the 496-wide layout; the host
# scatters the 16 column blocks to offsets R0[l] during the gather step.
N_ACT = OUT_SIDE * OUT_SIDE           # 256

# Output columns outside [OUT_LO, OUT_HI) are always zero.
OUT_LO = R0[0]                        # 15
OUT_HI = R0[OUT_SIDE - 1] + OUT_SIDE  # 376

_CACHE = {}


def build_program(rows: int = ROWS_PER_CORE, r: int = R, repeat: int = 1,
                  internal_io: bool = False, mode: str = "full"):
    """Build the per-core Bass program: x [rows, 1024] -> out [rows, 496].

    repeat > 1 wraps the whole body in a hardware For_i loop that redoes
    the identical pass `repeat` times — used only for benchmarking (the
    slope over `repeat` isolates on-device time from host overhead).

    internal_io=True replaces the I/O tensors with internal DRAM buffers
    (plus a dummy [1,1] external output) so benchmark calls skip the
    256 MiB host<->device transfer entirely. The instruction stream is
    identical to the real program.

    mode: "full" (real kernel) | "in_only" | "out_only" (DMA stream
    probes for benchmarking).
    """
    nc = bacc.Bacc("TRN2", target_bir_lowering=False, debug=False,
                   num_devices=N_CORES)
    f32 = mybir.dt.float32
    if internal_io:
        x = nc.dram_tensor("xbuf", [rows, N_FEAT], f32).ap()
        out = nc.dram_tensor("obuf", [rows, N_ACT], f32).ap()
        dummy = nc.dram_tensor("out", [1, 1], f32, kind="ExternalOutput").ap()
    else:
        x = nc.dram_tensor("x", [rows, N_FEAT], f32,
                           kind="ExternalInput").ap()
        out = nc.dram_tensor("out", [rows, N_ACT], f32,
                             kind="ExternalOutput").ap()

    # Chunk schedule: r-row supertiles. (A tail-split variant — ending
    # with r/2, r/4, r/4 chunks to shorten the end-of-pass compute tail —
    # measured no better on HW, so plain uniform chunks are used.)
    r_units = rows // P
    chunk_rs = [r] * (r_units // r)
    rest = r_units - sum(chunk_rs)
    if rest:
        chunk_rs.append(rest)
    assert sum(chunk_rs) == r_units

    # Per chunk: partition p holds rr consecutive batch rows.
    def x_view(row0, rr):
        return x[row0:row0 + P * rr].rearrange("(p r) m -> p (r m)",
                                               p=P, r=rr)

    def o_view(row0, rr):
        return out[row0:row0 + P * rr].rearrange("(p r) m -> p (r m)",
                                                 p=P, r=rr)

    with TileContext(nc) as tc:
        with tc.tile_pool(name="xin", bufs=3) as xin_pool, \
             tc.tile_pool(name="y1", bufs=2) as y1_pool, \
             tc.tile_pool(name="y2", bufs=3) as y2_pool:
            if internal_io:
                # zero-fill the internal input region once so the bench
                # never squares NaN/Inf garbage, and feed the dummy output
                zt = xin_pool.tile([P, r * N_FEAT], f32, tag="xt")
                nc.gpsimd.memset(zt[:], 0.0)
                row0 = 0
                for rr in chunk_rs:
                    nc.sync.dma_start(out=x_view(row0, rr),
                                      in_=zt[:, :rr * N_FEAT])
                    row0 += P * rr
                nc.sync.dma_start(out=dummy, in_=zt[:1, :1])

            def body():
                row0 = 0
                for t, rr in enumerate(chunk_rs):
                    if mode == "out_only":
                        nc.scalar.dma_start(out=o_view(row0, rr),
                                            in_=zt[:, :rr * N_ACT])
                        row0 += P * rr
                        continue
                    xt = xin_pool.tile([P, rr * N_FEAT], f32, tag="xt")
                    nc.sync.dma_start(out=xt[:], in_=x_view(row0, rr))
                    if mode == "in_only":
                        row0 += P * rr
                        continue

                    # square in place (elementwise, same AP — safe)
                    nc.scalar.activation(xt[:], xt[:],
                                         mybir.ActivationFunctionType.Square)

                    # pool over j: y1[p, 512rr], index = 512*row + 16*i + c
                    y1 = y1_pool.tile([P, rr * N_FEAT // 2], f32, tag="y1")
                    nc.vector.tensor_add(y1[:], xt[:, 0::2], xt[:, 1::2])

                    # pool over i: one dense add into y2 [p, rr*256]
                    # (y1 viewed [p, row, l, two, c]; y2 = even + odd i)
                    y1v = y1[:].rearrange("p (row l two c) -> p row l two c",
                                          row=rr, l=OUT_SIDE, two=2,
                                          c=OUT_SIDE)
                    y2 = y2_pool.tile([P, rr * N_ACT], f32, tag="y2")
                    y2v = y2[:].rearrange("p (row l c) -> p row l c",
                                          row=rr, l=OUT_SIDE, c=OUT_SIDE)
                    nc.vector.tensor_add(y2v, y1v[:, :, :, 0, :],
                                         y1v[:, :, :, 1, :])

                    # contiguous dense store, issued from the
                    # otherwise-idle GPSIMD engine (SWDGE) so its
                    # wait-for-DVE never stalls another sequencer
                    nc.gpsimd.dma_start(out=o_view(row0, rr), in_=y2[:])
                    row0 += P * rr

            if repeat == 1:
                body()
            else:
                with tc.For_i(0, repeat, 1):
                    body()

    nc.compile()
    return nc


def kernel(**inputs) -> np.ndarray:
    x = np.ascontiguousarray(inputs["input_state"], dtype=np.float32)
    assert x.shape == (BATCH, N_FEAT), x.shape

    if "nc" not in _CACHE:
        _CACHE["nc"] = build_program()
    nc = _CACHE["nc"]

    shards = [x[i * ROWS_PER_CORE:(i + 1) * ROWS_PER_CORE]
              for i in range(N_CORES)]
    in_maps = [{"x": s} for s in shards]
    res = run_bass_kernel_spmd(nc, in_maps, list(range(N_CORES)))

    # gather + unshard: scatter the dense 16-col blocks to R0[l] and
    # materialize the always-zero columns host-side
    compact = np.concatenate([res.results[i]["out"] for i in range(N_CORES)],
                             axis=0)
    full = np.zeros((BATCH, N_OUT), dtype=np.float32)
    for l in range(OUT_SIDE):
        full[:, R0[l]:R0[l] + OUT_SIDE] = \
            compact[:, l * OUT_SIDE:(l + 1) * OUT_SIDE]
    return full



# revision 8
# speedup vs baseline: 1.1347x; 1.1347x over previous
"""Trainium2 Bass kernel for nn_Custom_Final_Pooling_2D (segment_reduce).

Computes out = einsum("rn,bn->br", T, x*x) where T is the fixed binary
2x2-pooling selector built by the reference's build_pooling_matrix(32, 16):
  - T has shape [496, 1024]; only rows r0(l)+c are nonzero, where
    r0(l) = 31*l - l*(l+1)//2 + 15, for l, c in [0, 16).
  - Row r0(l)+c sums x[.., i*32+j]^2 over the 2x2 window
    i in {2l, 2l+1}, j in {2c, 2c+1}.

So the kernel is: square (ScalarE, in place), pairwise add along j
(VectorE, stride-2), pairwise add along i (VectorE) into the dense
[rows, 256] pool result stored as float16, then a contiguous DMA store
of that dense layout. The host upcasts to f32, scatters the 16 column
blocks to offsets r0(l) and materializes the 240 always-zero columns
while gathering. (Writing the 496- or 361-wide layouts on device moves
27-48% more bytes; strided partial-width stores measured ~1.9x slower
per byte than contiguous. f16 halves the store bytes again; the rel-err
cost is ~1e-4 against a 2e-2 budget.)

The chunk schedule tail-splits the final supertile (8,8,...,8,4,2,1,1
rows-per-partition) so the last load's dependent compute+store chain is
~1/8 the length: with HBM bandwidth saturated by the load stream, the
end-of-pass compute tail is the only exposed latency.

Data-parallel over 8 NeuronCores: batch dim sharded 65536 -> 8 x 8192.
"""

import numpy as np

import concourse.bacc as bacc
import concourse.mybir as mybir
from concourse.tile import TileContext
from concourse.bass_utils import run_bass_kernel_spmd

N_CORES = 8
BATCH = 65536
IMG = 32          # input image side
OUT_SIDE = 16     # pooled side
N_FEAT = IMG * IMG          # 1024
N_OUT = (2 * OUT_SIDE) * (2 * OUT_SIDE - 1) // 2  # 496
ROWS_PER_CORE = BATCH // N_CORES  # 8192

P = 128           # SBUF partitions
R = 8             # batch rows per partition per supertile
SUPER = P * R     # 1024 batch rows per supertile
N_TILES = ROWS_PER_CORE // SUPER  # 8

# Nonzero-row offsets of T: line l's 16 outputs live at columns
# r0(l) .. r0(l)+15 of the 496-wide output. Cols < 15 and >= 376 are
# always zero (as are the interior gaps); they stay at the memset value.
R0 = [31 * l - l * (l + 1) // 2 + 15 for l in range(OUT_SIDE)]

# The device writes the fully dense [rows, 256] pool output (line-major
# 16x16 blocks) — the exact nonzero values, contiguous, at full write
# bandwidth with 48% fewer bytes than the 496-wide layout; the host
# scatters the 16 column blocks to offsets R0[l] during the gather step.
N_ACT = OUT_SIDE * OUT_SIDE           # 256

# Output columns outside [OUT_LO, OUT_HI) are always zero.
OUT_LO = R0[0]                        # 15
OUT_HI = R0[OUT_SIDE - 1] + OUT_SIDE  # 376

_CACHE = {}


def build_program(rows: int = ROWS_PER_CORE, r: int = R, repeat: int = 1,
                  internal_io: bool = False, mode: str = "full"):
    """Build the per-core Bass program: x [rows, 1024] -> out [rows, 496].

    repeat > 1 wraps the whole body in a hardware For_i loop that redoes
    the identical pass `repeat` times — used only for benchmarking (the
    slope over `repeat` isolates on-device time from host overhead).

    internal_io=True replaces the I/O tensors with internal DRAM buffers
    (plus a dummy [1,1] external output) so benchmark calls skip the
    256 MiB host<->device transfer entirely. The instruction stream is
    identical to the real program.

    mode: "full" (real kernel) | "in_only" | "out_only" (DMA stream
    probes for benchmarking).
    """
    nc = bacc.Bacc("TRN2", target_bir_lowering=False, debug=False,
                   num_devices=N_CORES)
    f32 = mybir.dt.float32
    f16 = mybir.dt.float16
    if internal_io:
        x = nc.dram_tensor("xbuf", [rows, N_FEAT], f32).ap()
        out = nc.dram_tensor("obuf", [rows, N_ACT], f16).ap()
        dummy = nc.dram_tensor("out", [1, 1], f32, kind="ExternalOutput").ap()
    else:
        x = nc.dram_tensor("x", [rows, N_FEAT], f32,
                           kind="ExternalInput").ap()
        out = nc.dram_tensor("out", [rows, N_ACT], f16,
                             kind="ExternalOutput").ap()

    # Chunk schedule: r-row supertiles, with the final supertile split
    # into descending halves (8 -> 4,2,1,1) so the last load's dependent
    # square->add->add->store chain is ~8x shorter. The load stream
    # saturates HBM for the whole pass, so that chain is the only
    # latency the schedule exposes.
    r_units = rows // P
    chunk_rs = [r] * (r_units // r)
    rest = r_units - sum(chunk_rs)
    if rest:
        chunk_rs.append(rest)
    if len(chunk_rs) > 1 and chunk_rs[-1] > 1:
        last = chunk_rs.pop()
        while last > 1:
            half = last // 2
            chunk_rs.append(half)
            last -= half
        chunk_rs.append(last)
    assert sum(chunk_rs) == r_units

    # Per chunk: partition p holds rr consecutive batch rows.
    def x_view(row0, rr):
        return x[row0:row0 + P * rr].rearrange("(p r) m -> p (r m)",
                                               p=P, r=rr)

    def o_view(row0, rr):
        return out[row0:row0 + P * rr].rearrange("(p r) m -> p (r m)",
                                                 p=P, r=rr)

    with TileContext(nc) as tc:
        with tc.tile_pool(name="xin", bufs=3) as xin_pool, \
             tc.tile_pool(name="y1", bufs=2) as y1_pool, \
             tc.tile_pool(name="y2", bufs=3) as y2_pool:
            if internal_io:
                # zero-fill the internal input region once so the bench
                # never squares NaN/Inf garbage, and feed the dummy output
                zt = xin_pool.tile([P, r * N_FEAT], f32, tag="xt")
                nc.gpsimd.memset(zt[:], 0.0)
                ztb = y2_pool.tile([P, r * N_ACT], f16, tag="y2")
                nc.gpsimd.memset(ztb[:], 0.0)
                row0 = 0
                for rr in chunk_rs:
                    nc.sync.dma_start(out=x_view(row0, rr),
                                      in_=zt[:, :rr * N_FEAT])
                    row0 += P * rr
                nc.sync.dma_start(out=dummy, in_=zt[:1, :1])

            def body():
                row0 = 0
                for t, rr in enumerate(chunk_rs):
                    if mode == "out_only":
                        nc.scalar.dma_start(out=o_view(row0, rr),
                                            in_=ztb[:, :rr * N_ACT])
                        row0 += P * rr
                        continue
                    xt = xin_pool.tile([P, rr * N_FEAT], f32, tag="xt")
                    nc.sync.dma_start(out=xt[:], in_=x_view(row0, rr))
                    if mode == "in_only":
                        row0 += P * rr
                        continue

                    # square in place (elementwise, same AP — safe)
                    nc.scalar.activation(xt[:], xt[:],
                                         mybir.ActivationFunctionType.Square)

                    # pool over j: y1[p, 512rr], index = 512*row + 16*i + c
                    y1 = y1_pool.tile([P, rr * N_FEAT // 2], f32, tag="y1")
                    nc.vector.tensor_add(y1[:], xt[:, 0::2], xt[:, 1::2])

                    # pool over i: one dense add into y2 [p, rr*256]
                    # (y1 viewed [p, row, l, two, c]; y2 = even + odd i)
                    y1v = y1[:].rearrange("p (row l two c) -> p row l two c",
                                          row=rr, l=OUT_SIDE, two=2,
                                          c=OUT_SIDE)
                    y2 = y2_pool.tile([P, rr * N_ACT], f16, tag="y2")
                    y2v = y2[:].rearrange("p (row l c) -> p row l c",
                                          row=rr, l=OUT_SIDE, c=OUT_SIDE)
                    nc.vector.tensor_add(y2v, y1v[:, :, :, 0, :],
                                         y1v[:, :, :, 1, :])

                    # contiguous dense store, issued from the
                    # otherwise-idle GPSIMD engine (SWDGE) so its
                    # wait-for-DVE never stalls another sequencer
                    nc.gpsimd.dma_start(out=o_view(row0, rr), in_=y2[:])
                    row0 += P * rr

            if repeat == 1:
                body()
            else:
                with tc.For_i(0, repeat, 1):
                    body()

    nc.compile()
    return nc


def kernel(**inputs) -> np.ndarray:
    x = np.ascontiguousarray(inputs["input_state"], dtype=np.float32)
    assert x.shape == (BATCH, N_FEAT), x.shape

    if "nc" not in _CACHE:
        _CACHE["nc"] = build_program()
    nc = _CACHE["nc"]

    shards = [x[i * ROWS_PER_CORE:(i + 1) * ROWS_PER_CORE]
              for i in range(N_CORES)]
    in_maps = [{"x": s} for s in shards]
    res = run_bass_kernel_spmd(nc, in_maps, list(range(N_CORES)))

    # gather + unshard: upcast the f16 device output, scatter the dense
    # 16-col blocks to R0[l] and materialize the always-zero columns
    compact = np.concatenate([np.asarray(res.results[i]["out"])
                              for i in range(N_CORES)], axis=0)
    compact = compact.astype(np.float32)
    full = np.zeros((BATCH, N_OUT), dtype=np.float32)
    for l in range(OUT_SIDE):
        full[:, R0[l]:R0[l] + OUT_SIDE] = \
            compact[:, l * OUT_SIDE:(l + 1) * OUT_SIDE]
    return full



# revision 15
# speedup vs baseline: 1.2185x; 1.0739x over previous
"""Trainium2 Bass kernel for nn_Custom_Final_Pooling_2D (segment_reduce).

Computes out = einsum("rn,bn->br", T, x*x) where T is the fixed binary
2x2-pooling selector built by the reference's build_pooling_matrix(32, 16):
  - T has shape [496, 1024]; only rows r0(l)+c are nonzero, where
    r0(l) = 31*l - l*(l+1)//2 + 15, for l, c in [0, 16).
  - Row r0(l)+c sums x[.., i*32+j]^2 over the 2x2 window
    i in {2l, 2l+1}, j in {2c, 2c+1}.

So the kernel is: square (ScalarE, in place), pairwise add along j
(VectorE, stride-2), pairwise add along i (VectorE) into the dense
[rows, 256] pool result stored as float16, then a contiguous DMA store
of that dense layout. The host upcasts to f32, scatters the 16 column
blocks to offsets r0(l) and materializes the 240 always-zero columns
while gathering. (Writing the 496- or 361-wide layouts on device moves
27-48% more bytes; strided partial-width stores measured ~1.9x slower
per byte than contiguous. f16 halves the store bytes again; the rel-err
cost is ~1e-4 against a 2e-2 budget.)

The chunk schedule tail-splits the final supertile (8,8,...,8,4,2,1,1
rows-per-partition) so the last load's dependent compute+store chain is
~1/8 the length: with HBM bandwidth saturated by the load stream, the
end-of-pass compute tail is the only exposed latency.

Data-parallel over 8 NeuronCores: batch dim sharded 65536 -> 8 x 8192.
"""

import numpy as np

import concourse.bacc as bacc
import concourse.mybir as mybir
from concourse.tile import TileContext
from concourse.bass_utils import run_bass_kernel_spmd

N_CORES = 8
BATCH = 65536
IMG = 32          # input image side
OUT_SIDE = 16     # pooled side
N_FEAT = IMG * IMG          # 1024
N_OUT = (2 * OUT_SIDE) * (2 * OUT_SIDE - 1) // 2  # 496
ROWS_PER_CORE = BATCH // N_CORES  # 8192

P = 128           # SBUF partitions
R = 8             # batch rows per partition per supertile
SUPER = P * R     # 1024 batch rows per supertile
N_TILES = ROWS_PER_CORE // SUPER  # 8

# Nonzero-row offsets of T: line l's 16 outputs live at columns
# r0(l) .. r0(l)+15 of the 496-wide output. Cols < 15 and >= 376 are
# always zero (as are the interior gaps); they stay at the memset value.
R0 = [31 * l - l * (l + 1) // 2 + 15 for l in range(OUT_SIDE)]

# The device writes the fully dense [rows, 256] pool output (line-major
# 16x16 blocks) — the exact nonzero values, contiguous, at full write
# bandwidth with 48% fewer bytes than the 496-wide layout; the host
# scatters the 16 column blocks to offsets R0[l] during the gather step.
N_ACT = OUT_SIDE * OUT_SIDE           # 256

# Output columns outside [OUT_LO, OUT_HI) are always zero.
OUT_LO = R0[0]                        # 15
OUT_HI = R0[OUT_SIDE - 1] + OUT_SIDE  # 376

_CACHE = {}


def build_program(rows: int = ROWS_PER_CORE, r: int = R, repeat: int = 1,
                  internal_io: bool = False, mode: str = "full"):
    """Build the per-core Bass program: x [rows, 1024] -> out [rows, 496].

    repeat > 1 wraps the whole body in a hardware For_i loop that redoes
    the identical pass `repeat` times — used only for benchmarking (the
    slope over `repeat` isolates on-device time from host overhead).

    internal_io=True replaces the I/O tensors with internal DRAM buffers
    (plus a dummy [1,1] external output) so benchmark calls skip the
    256 MiB host<->device transfer entirely. The instruction stream is
    identical to the real program.

    mode: "full" (real kernel) | "in_only" | "out_only" | "inout"
    (loads + dep-free stores on separate queues, overlap allowed) |
    "inout_ser" (loads then stores on one queue, strictly serialized)
    — DMA stream probes for benchmarking.
    """
    nc = bacc.Bacc("TRN2", target_bir_lowering=False, debug=False,
                   num_devices=N_CORES)
    f32 = mybir.dt.float32
    f16 = mybir.dt.float16
    if internal_io:
        x = nc.dram_tensor("xbuf", [rows, N_FEAT], f32).ap()
        out = nc.dram_tensor("obuf", [rows, N_ACT], f16).ap()
        dummy = nc.dram_tensor("out", [1, 1], f32, kind="ExternalOutput").ap()
    else:
        x = nc.dram_tensor("x", [rows, N_FEAT], f32,
                           kind="ExternalInput").ap()
        out = nc.dram_tensor("out", [rows, N_ACT], f16,
                             kind="ExternalOutput").ap()

    # Chunk schedule: uniform r-row supertiles. (Tail-split variants
    # measured SLOWER on the load stream — 98.4 vs 95.7 us — and the
    # deferred-store schedule below removes the compute-tail motivation.)
    r_units = rows // P
    chunk_rs = [r] * (r_units // r)
    rest = r_units - sum(chunk_rs)
    if rest:
        chunk_rs.append(rest)
    assert sum(chunk_rs) == r_units

    # Per chunk: partition p holds rr consecutive batch rows.
    def x_view(row0, rr):
        return x[row0:row0 + P * rr].rearrange("(p r) m -> p (r m)",
                                               p=P, r=rr)

    def o_view(row0, rr):
        return out[row0:row0 + P * rr].rearrange("(p r) m -> p (r m)",
                                                 p=P, r=rr)

    with TileContext(nc) as tc:
        with tc.tile_pool(name="xin", bufs=3) as xin_pool, \
             tc.tile_pool(name="y1", bufs=2) as y1_pool, \
             tc.tile_pool(name="y2", bufs=1) as y2_pool:
            if internal_io:
                # zero-fill the internal input region once so the bench
                # never squares NaN/Inf garbage, and feed the dummy output
                zt = xin_pool.tile([P, r * N_FEAT], f32, tag="xt")
                nc.gpsimd.memset(zt[:], 0.0)
                ztb = y2_pool.tile([P, r * N_ACT], f16, tag="y2z")
                nc.gpsimd.memset(ztb[:], 0.0)
                row0 = 0
                for rr in chunk_rs:
                    nc.sync.dma_start(out=x_view(row0, rr),
                                      in_=zt[:, :rr * N_FEAT])
                    row0 += P * rr
                nc.sync.dma_start(out=dummy, in_=zt[:1, :1])

            def body():
                row0 = 0
                if mode in ("inout", "inout_ser"):
                    st_engine = nc.gpsimd if mode == "inout" else nc.sync
                    for rr in chunk_rs:
                        xt = xin_pool.tile([P, rr * N_FEAT], f32, tag="xt")
                        nc.sync.dma_start(out=xt[:], in_=x_view(row0, rr))
                        row0 += P * rr
                    row0 = 0
                    for rr in chunk_rs:
                        st_engine.dma_start(out=o_view(row0, rr),
                                            in_=ztb[:, :rr * N_ACT])
                        row0 += P * rr
                    return
                stores = []
                for t, rr in enumerate(chunk_rs):
                    if mode == "out_only":
                        nc.scalar.dma_start(out=o_view(row0, rr),
                                            in_=ztb[:, :rr * N_ACT])
                        row0 += P * rr
                        continue
                    xt = xin_pool.tile([P, rr * N_FEAT], f32, tag="xt")
                    nc.sync.dma_start(out=xt[:], in_=x_view(row0, rr))
                    if mode == "in_only":
                        row0 += P * rr
                        continue

                    # square in place (elementwise, same AP — safe)
                    nc.scalar.activation(xt[:], xt[:],
                                         mybir.ActivationFunctionType.Square)

                    # pool over j: y1[p, 512rr], index = 512*row + 16*i + c
                    y1 = y1_pool.tile([P, rr * N_FEAT // 2], f32, tag="y1")
                    nc.vector.tensor_add(y1[:], xt[:, 0::2], xt[:, 1::2])

                    # pool over i: one dense add into y2 [p, rr*256]
                    # (y1 viewed [p, row, l, two, c]; y2 = even + odd i)
                    y1v = y1[:].rearrange("p (row l two c) -> p row l two c",
                                          row=rr, l=OUT_SIDE, two=2,
                                          c=OUT_SIDE)
                    # each chunk gets its own live y2 buffer (tag per
                    # chunk) so the store can be deferred past all loads
                    y2 = y2_pool.tile([P, rr * N_ACT], f16, tag=f"y2_{t}")
                    y2v = y2[:].rearrange("p (row l c) -> p row l c",
                                          row=rr, l=OUT_SIDE, c=OUT_SIDE)
                    nc.vector.tensor_add(y2v, y1v[:, :, :, 0, :],
                                         y1v[:, :, :, 1, :])
                    stores.append((row0, rr, y2))
                    row0 += P * rr

                # All stores issued on the SYNC queue AFTER every load:
                # FIFO order serializes writes behind the read stream.
                # Measured: overlapped mixed read/write traffic runs at
                # 327 GB/s, strictly serialized streams at 351 GB/s —
                # read/write turnaround on HBM makes overlap a net loss.
                # By the time the loads drain, every chunk's compute but
                # the last is done, so the stores stream back-to-back.
                for row0_s, rr, y2 in stores:
                    nc.sync.dma_start(out=o_view(row0_s, rr), in_=y2[:])

            if repeat == 1:
                body()
            else:
                with tc.For_i(0, repeat, 1):
                    body()

    nc.compile()
    return nc


def kernel(**inputs) -> np.ndarray:
    x = np.ascontiguousarray(inputs["input_state"], dtype=np.float32)
    assert x.shape == (BATCH, N_FEAT), x.shape

    if "nc" not in _CACHE:
        _CACHE["nc"] = build_program()
    nc = _CACHE["nc"]

    shards = [x[i * ROWS_PER_CORE:(i + 1) * ROWS_PER_CORE]
              for i in range(N_CORES)]
    in_maps = [{"x": s} for s in shards]
    res = run_bass_kernel_spmd(nc, in_maps, list(range(N_CORES)))

    # gather + unshard: upcast the f16 device output, scatter the dense
    # 16-col blocks to R0[l] and materialize the always-zero columns
    compact = np.concatenate([np.asarray(res.results[i]["out"])
                              for i in range(N_CORES)], axis=0)
    compact = compact.astype(np.float32)
    full = np.zeros((BATCH, N_OUT), dtype=np.float32)
    for l in range(OUT_SIDE):
        full[:, R0[l]:R0[l] + OUT_SIDE] = \
            compact[:, l * OUT_SIDE:(l + 1) * OUT_SIDE]
    return full

